# revision 14
# baseline (speedup 1.0000x reference)
"""Trainium2 Bass kernel for the attention-scoring module:

    out[b, s] = softmax_s( (enc[b] @ W.T + bias) @ h[b] )

Math: the bias term contributes a constant per (b, :) row, which cancels in
the softmax, and the two contractions reassociate:

    energies[b, s] = enc[b, s, :] . v[b]   with   v[b] = h[b] @ W

Sharding: data-parallel over batch - one batch per NeuronCore (B == 8 cores).

This revision streams enc (and W, h) as float16: the softmax tolerates the
quantization (measured rel-l2 ~6e-4 on the harness inputs, vs the 2e-2
gate), and the serializing resource is the DMA bus, so halving the bytes
halves the stream time (16 MiB -> 8 MiB, ~46.6us -> ~23.3us of DMA busy).

At fp16 arrival rates (~364 ns per 128-row block) no single engine keeps up
with the dot products (the fused DVE multiply+row-sum runs at 1x,
~612 ns/block), so the 64 blocks are split across three paths:

  - 'D': DVE fused scalar_tensor_tensor multiply+row-sum  (~612 ns/block)
  - 'A': DVE tensor_tensor multiply in fp16 2x mode (~332 ns) + ACT
         Copy-activation with accum_out row-sum            (~810 ns/block)
  - 'G': GPSIMD tensor_tensor multiply (~1110 ns) + the same ACT reduce
         (the Pool engine cannot run the fused scalar_tensor_tensor, and
         XBAR-transposed loads for a PE path serialize against regular
         DMAs, so both alternatives lose).

Softmax is incremental: shift from the first SHIFT_C columns mid-stream,
staged exp+row-sum, PE transposes of the prob columns, and a short tail
(exp of the last columns + sum + reciprocal + scale + one output DMA).
"""

from contextlib import ExitStack

import numpy as np

import concourse.tile as tile
from concourse import bacc, mybir
from concourse import bass_isa
from concourse.bass_utils import run_bass_kernel_spmd
from concourse.masks import make_identity

B, S, H = 8, 8192, 512
N_CORES = 8
P = 128
N_COLS = S // P  # 64 energy columns, E[p, t] = energy(s = t*128 + p)
F32 = mybir.dt.float32
F16 = mybir.dt.float16
ALU = mybir.AluOpType
ACTF = mybir.ActivationFunctionType
AXX = mybir.AxisListType.X

SHIFT_C = 16  # softmax shift comes from the first 16 columns, mid-stream
EC = 32       # stage-1 exp/transpose boundary (32-aligned partition offsets)
MC = 62       # second exp stage covers cols EC..MC

# enc DMA grouping: big groups early, singles late so the tail only waits
# on one 128-row block.
DMA_GROUPS = [2] * 31 + [1] * 2

# Per-block engine costs (ns) for the offline greedy scheduler.
COST_DVE_FUSED = 612.0
COST_DVE_MULT = 332.0
COST_ACT_REDUCE = 830.0
COST_GP_MULT = 1110.0


def _sim_schedule(assign, dma_groups):
    """Small discrete-event model of the kernel schedule: in-order engine
    queues, DMA arrivals, cross-engine deps.  Returns estimated end time."""
    SEM = 30.0
    t = 1970.0 + 3.0 + 1456.0  # h + W chunk DMAs
    arrivals = []
    for g in dma_groups:
        t += g * 364.0
        arrivals += [t + 900.0] * g
    v_ready = 5500.0
    free = {"D": v_ready, "A": v_ready, "G": v_ready}
    edone = [0.0] * N_COLS  # time E[:, b] is written
    negM = None
    rs1_done = 0.0

    def run(eng, ready, cost):
        s = max(free[eng], ready)
        free[eng] = s + cost
        return s + cost

    for b in range(N_COLS):
        p = assign[b]
        if p == "D":
            edone[b] = run("D", arrivals[b], 612.0)
        elif p == "A":
            m = run("D", arrivals[b], 332.0)
            edone[b] = run("A", m + SEM, 810.0)
        else:
            m = run("G", arrivals[b], 1110.0)
            edone[b] = run("A", m + SEM, 810.0)
        if b + 1 == SHIFT_C:
            mx = max(edone[:SHIFT_C]) + SEM
            r = run("D", mx, 110.0)
            g = run("G", r + SEM, 250.0)
            negM = run("D", g + SEM, 62.0)
        if b + 1 == EC:
            mx = max(max(edone[:EC]) + SEM, negM + SEM)
            rs1_done = run("A", mx, 800.0)
        if b + 1 == MC:
            mx = max(max(edone[:MC]) + SEM, negM + SEM)
            s2 = run("A", mx, 1000.0)
            run("D", max(s2 + SEM, rs1_done + SEM), 62.0)
    # tail
    e_tail = run("A", max(edone[MC : N_COLS - 1]) + SEM, 210.0)
    e63 = run("A", max(edone[N_COLS - 1] + SEM, e_tail), 210.0)
    red = run("D", e63 + SEM, 75.0)
    allr = run("G", red + SEM, 250.0)
    rec = run("D", allr + SEM, 62.0)
    s1 = run("D", rec, 193.0)
    s2 = run("D", s1, 193.0)
    return s2 + SEM + 625.0 + 650.0 + 91.0 + 800.0


def _assign_engines():
    """Balanced base assignment improved by local search against the
    schedule model."""
    import random

    rng = random.Random(1234)
    # balanced seed: mostly D with G every ~3rd block and sparse A
    base = []
    for b in range(N_COLS):
        r = b % 16
        if r in (1, 4, 7, 10, 13):
            base.append("G")
        elif r == 14:
            base.append("A")
        else:
            base.append("D")
    base[N_COLS - 1] = "D"
    best, best_t = base[:], _sim_schedule(base, DMA_GROUPS)
    for _ in range(20000):
        cand = best[:]
        for _ in range(rng.randint(1, 2)):
            i = rng.randrange(N_COLS - 1)
            cand[i] = rng.choice("DAG".replace(cand[i], ""))
        t = _sim_schedule(cand, DMA_GROUPS)
        if t <= best_t:
            best, best_t = cand, t
    return best


def _build_kernel():
    nc = bacc.Bacc("TRN2", target_bir_lowering=False, debug=False)
    enc = nc.dram_tensor("enc", [S, H], F16, kind="ExternalInput")
    hvec = nc.dram_tensor("hvec", [1, H], F16, kind="ExternalInput")
    Wmat = nc.dram_tensor("W", [H, H], F16, kind="ExternalInput")
    out = nc.dram_tensor("out", [S], F32, kind="ExternalOutput")

    engine_of = _assign_engines()

    with ExitStack() as ctx:
        tc = ctx.enter_context(tile.TileContext(nc))
        consts = ctx.enter_context(tc.tile_pool(name="consts", bufs=1))
        small = ctx.enter_context(tc.tile_pool(name="small", bufs=1))
        psum = ctx.enter_context(tc.tile_pool(name="psum", bufs=1, space="PSUM"))

        # Big flat SBUF regions (raw tensors: no pool-close drain and no
        # buffer recycling, so block compute carries no WAR waits).
        enc_t = ctx.enter_context(nc.sbuf_tensor("enc_all", [P, N_COLS, H], F16))
        enc_all = enc_t.ap()
        prod_t = ctx.enter_context(nc.sbuf_tensor("prods", [P, N_COLS, H], F16))
        prods = prod_t.ap()
        final_t = ctx.enter_context(nc.sbuf_tensor("final", [N_COLS, P], F32))
        final = final_t.ap()

        # Constants (identity first: the PE warm-up waits on it).
        identity = consts.tile([P, P], F32)
        make_identity(nc, identity[:])
        ones_row16 = consts.tile([1, P], F16)
        nc.gpsimd.memset(ones_row16[:], 1.0)
        one11 = consts.tile([1, 1], F32)
        nc.gpsimd.memset(one11[:], 1.0)

        # ---- input DMA queue: hvec, W (4 chunks), then enc groups ----
        hrow = small.tile([1, H], F16)
        nc.sync.dma_start(hrow[:], hvec.ap())
        W_sb = small.tile([P, 4, H], F16)
        for half in range(2):
            nc.sync.dma_start(
                W_sb[:, 2 * half : 2 * half + 2, :],
                Wmat.ap()[half * 2 * P : (half + 1) * 2 * P, :].rearrange(
                    "(c p) h -> p c h", c=2, p=P
                ),
            )
        blk = 0
        for g in DMA_GROUPS:
            nc.sync.dma_start(
                enc_all[:, blk : blk + g, :],
                enc.ap()[blk * P : (blk + g) * P, :].rearrange(
                    "(c p) h -> p c h", c=g, p=P
                ),
            )
            blk += g

        # Trigger the ACT exp table load at t=0 instead of in the tail.
        dummy_act = small.tile([1, 1], F32)
        nc.scalar.activation(dummy_act[:], one11[:], ACTF.Exp, bias=0.0, scale=1.0)

        # PE p-state warm-up: keep the PE continuously busy until hrow
        # lands so the v matmuls run at a ramped clock.
        warm = psum.tile([P, P], F32, tag="pwarm")
        for i in range(7):
            nc.tensor.transpose(warm[:], identity[:], identity[:])

        # ---- v = h @ W, broadcast to all 128 partitions (fp16 pipeline) ----
        # Stage 1: hTb_c[m, n] = h[c*128+m] for all n (hrow-chunk stationary
        # x ones_row moving); 4 distinct PSUM tags so nothing ping-pongs.
        # Copies PSUM->SBUF alternate ACT/DVE to halve the serial chain.
        hT_sb = []
        for c in range(4):
            hT_ps = psum.tile([P, P], F32, tag=f"ph{c}", name=f"hT_ps{c}")
            nc.tensor.matmul(
                hT_ps[:],
                hrow[:1, c * P : (c + 1) * P],
                ones_row16[:],
                start=True,
                stop=True,
            )
            ht = small.tile([P, P], F16, tag=f"ht{c}", name=f"ht{c}")
            hT_sb.append(ht)
            if c % 2 == 0:
                nc.scalar.copy(ht[:], hT_ps[:])
            else:
                nc.vector.tensor_copy(ht[:], hT_ps[:])
        for i in range(3):
            nc.tensor.transpose(warm[:], identity[:], identity[:])
        v_bc_ps = psum.tile([P, H], F32, tag="vbc")
        for c in range(4):
            nc.tensor.matmul(
                v_bc_ps[:],
                hT_sb[c][:],
                W_sb[:, c, :],
                start=(c == 0),
                stop=(c == 3),
            )
        # v copy split across ACT and DVE so the consumers start sooner.
        v_sb = small.tile([P, H], F16)
        nc.scalar.copy(v_sb[:, : H // 2], v_bc_ps[:, : H // 2])
        nc.vector.tensor_copy(v_sb[:, H // 2 :], v_bc_ps[:, H // 2 :])

        # ---- main loop: stream enc, dot products split across 3 engines ----
        E = small.tile([P, N_COLS], F32)
        E63z = small.tile([P, 1], F32)  # dedicated final column
        P_exp = small.tile([P, N_COLS + 1], F32)  # col 64 holds rs12
        rs1 = small.tile([P, 1], F32)
        negM_sb = small.tile([P, 1], F32)
        probsT_ps = psum.tile([EC, P], F32, tag="ph0")
        probsT23_ps = psum.tile([N_COLS - EC, P], F32, tag="vbc")

        def emit_shift_chain():
            m_col = small.tile([P, 1], F32)
            nc.vector.tensor_reduce(
                m_col[:], E[:, :SHIFT_C], axis=AXX, op=ALU.max
            )
            M_bc = small.tile([P, 1], F32)
            nc.gpsimd.partition_all_reduce(
                M_bc[:], m_col[:], P, bass_isa.ReduceOp.max
            )
            nc.vector.tensor_scalar_mul(negM_sb[:], M_bc[:], -1.0)

        def emit_stage1_chain():
            nc.scalar.activation(
                P_exp[:, :EC],
                E[:, :EC],
                ACTF.Exp,
                bias=negM_sb[:],
                scale=1.0,
                accum_out=rs1[:],
            )
            nc.tensor.transpose(probsT_ps[:], P_exp[:, :EC], identity[:])

        def emit_mid_chain():
            rs2 = small.tile([P, 1], F32)
            nc.scalar.activation(
                P_exp[:, EC:MC],
                E[:, EC:MC],
                ACTF.Exp,
                bias=negM_sb[:],
                scale=1.0,
                accum_out=rs2[:],
            )
            nc.vector.tensor_add(P_exp[:, N_COLS : N_COLS + 1], rs1[:], rs2[:])

        for t in range(N_COLS):
            ch = enc_all[:, t, :]
            eng = engine_of[t]
            acc = E63z[:] if t == N_COLS - 1 else E[:, t : t + 1]
            pr = prods[:, t, :]
            if eng == "D":
                nc.vector.scalar_tensor_tensor(
                    out=pr, in0=ch, scalar=1.0, in1=v_sb[:],
                    op0=ALU.bypass, op1=ALU.mult, accum_out=acc,
                )
            elif eng == "G":
                nc.gpsimd.tensor_tensor(pr, ch, v_sb[:], op=ALU.mult)
                nc.scalar.activation(
                    pr, pr, ACTF.Copy, bias=0.0, scale=1.0, accum_out=acc,
                )
            else:  # 'A': fp16 2x multiply on DVE, reduce on ACT
                nc.vector.tensor_tensor(pr, ch, v_sb[:], op=ALU.mult)
                nc.scalar.activation(
                    pr, pr, ACTF.Copy, bias=0.0, scale=1.0, accum_out=acc,
                )
            if t + 1 == SHIFT_C:
                emit_shift_chain()
            if t + 1 == EC:
                emit_stage1_chain()
            if t + 1 == MC:
                emit_mid_chain()

        # ---- softmax tail: columns MC..63 ----
        nc.scalar.activation(
            P_exp[:, MC : N_COLS - 1],
            E[:, MC : N_COLS - 1],
            ACTF.Exp,
            bias=negM_sb[:],
            scale=1.0,
        )
        nc.scalar.activation(
            P_exp[:, N_COLS - 1 : N_COLS],
            E63z[:],
            ACTF.Exp,
            bias=negM_sb[:],
            scale=1.0,
        )
        nc.tensor.transpose(probsT23_ps[:], P_exp[:, EC:N_COLS], identity[:])
        rs_tot = small.tile([P, 1], F32)
        nc.vector.tensor_reduce(
            rs_tot[:], P_exp[:, MC : N_COLS + 1], axis=AXX, op=ALU.add
        )
        S_bc = small.tile([P, 1], F32)
        nc.gpsimd.partition_all_reduce(S_bc[:], rs_tot[:], P, bass_isa.ReduceOp.add)
        SinvB = small.tile([N_COLS, 1], F32)
        nc.vector.reciprocal(SinvB[:], S_bc[:N_COLS, :])
        # scale straight out of PSUM (SinvB entries identical -> base-0 ok)
        nc.vector.tensor_scalar_mul(
            final[EC:, :], probsT23_ps[:], SinvB[: N_COLS - EC, :]
        )
        nc.scalar.activation(
            final[:EC, :], probsT_ps[:], ACTF.Copy, bias=0.0,
            scale=SinvB[:EC, :],
        )
        nc.sync.dma_start(out.ap().rearrange("(t p) -> t p", p=P), final)

    nc.compile()
    return nc


_NC_CACHE = {}


def kernel(hidden, encoder_outputs, W, b):
    """Full (unsharded) inputs in, full output out; 8-core SPMD inside."""
    if "nc" not in _NC_CACHE:
        _NC_CACHE["nc"] = _build_kernel()
    nc = _NC_CACHE["nc"]

    hidden = np.asarray(hidden)
    enc16 = np.ascontiguousarray(np.asarray(encoder_outputs).astype(np.float16))
    W16 = np.ascontiguousarray(np.asarray(W).astype(np.float16))
    in_maps = [
        {
            "enc": enc16[c],
            "hvec": np.ascontiguousarray(
                hidden[0, c][None, :].astype(np.float16)
            ),
            "W": W16,
        }
        for c in range(N_CORES)
    ]
    res = run_bass_kernel_spmd(nc, in_maps, core_ids=list(range(N_CORES)))
    return np.stack([res.results[c]["out"] for c in range(N_CORES)], axis=0).astype(
        np.float32
    )
